# revision 1
# baseline (speedup 1.0000x reference)
"""3-layer GCN (GCNConv x3) on 8 Trainium2 NeuronCores.

Strategy (node-partitioned, PE scatter-add), V2:
  - Nodes are block-partitioned across the 8 cores by destination id
    (core c owns rows [c*OWN, (c+1)*OWN)).
  - Per layer: each core transforms its own node rows (H @ W, scaled by
    dis = deg^-1/2) into a packed bf16 table shard, the shards are
    AllGathered into a full bf16 node table in each core's DRAM, then
    each core aggregates messages for its own destinations:
      * the table is gathered PAIR-wise: dma_gather needs >=256B
        descriptors, and a bf16 row is only 128B, so each descriptor
        fetches the 256B pair (h[2i], h[2i+1]) and the aggregation
        selects the correct half per edge,
      * per-edge pair-rows are fetched with dma_gather (int16 pair
        indices; sources bucketed into two 12500-pair table segments),
      * scaled+merged into per-destination-block PSUM accumulators via
        TWO selection-matrix matmuls per chunk on the tensor engine
        (even-source edges contract against msg cols 0:64, odd-source
        edges against cols 64:128; sel_even/sel_odd are built in one
        batched DVE is_equal against an iota-columns constant, from a
        host-packed dstloc2 array that masks the wrong-parity slot
        with 255).
  - Epilogue (batched per layer): psum_lo + psum_hi + t_own, * dis[dst],
    + bias (+ relu) in a handful of whole-layer DVE ops.
  - norm = dis[src]*dis[dst] is folded as: table rows pre-scaled by
    dis[src], aggregated output post-scaled by dis[dst]; no per-edge
    scaling needed.

The host side only does graph partitioning / index packing (sorting,
bucketing, degree counts); all tensor math runs on the NeuronCores.
"""

import sys

sys.path.insert(0, "/opt/trn_rl_repo")

import numpy as np
import ml_dtypes

N_NODES = 50000
N_CORES = 8
OWN = N_NODES // N_CORES  # 6250
P = 128
NBLK = (OWN + P - 1) // P  # 49
LAST_ROWS = OWN - (NBLK - 1) * P  # 106
F_IN = 128
F_TAB = 64  # table width, all layers (layer-3 W padded 32->64)
F_OUT = 32
SEG_BOUND = 25000  # int16-safe source bucketing (pair ids < 12500)
G_SEL = 16  # chunks per DVE sel-build op
G_CALL = 48  # chunks per dma_gather call
N_BANK = (NBLK + 7) // 8  # PSUM banks for aggregation (7)
SINGLE_PACKET = False  # single-packet mode hangs SDMA on >1k-descriptor gathers
SKIP_AG = False  # ablation: drop the AllGather collectives
N_QUEUES = 4  # SWDGE queues; gather calls round-robin across them
DMA_SCRATCH = 16384  # dynamic-DMA scratch bytes (ring = /16 descriptors)
MSG_BUFS = 3  # in-flight gather destination tiles
SIM_1CORE = False  # build single-core (no collective) variant for TimelineSim

BF16 = ml_dtypes.bfloat16


def _wrap_idx(flat_idx):
    """int16 gather-index layout: idx j at [j%16, j//16], replicated x8."""
    w = flat_idx.astype(np.int16).reshape(-1, 16).T  # [16, T*8]
    return np.ascontiguousarray(np.tile(w, (8, 1)))  # [128, T*8]


def _greedy_assign(deg_lo, deg_hi):
    """Greedy vector bin-packing of nodes into (core, block) bins by
    per-segment incoming-edge load. Returns perm (old id -> new id)."""
    tot = deg_lo + deg_hi
    order = np.argsort(-tot, kind="stable")
    nbins = N_CORES * NBLK
    cap = np.zeros(nbins, dtype=np.int64)
    load = np.zeros((nbins, 2), dtype=np.float64)
    capacity = np.full(nbins, P, dtype=np.int64)
    capacity[NBLK - 1 :: NBLK] = LAST_ROWS
    perm = np.empty(N_NODES, dtype=np.int64)
    score = load.max(axis=1) * 4096 + load.sum(axis=1)
    for n in order:
        b = int(np.argmin(score))
        c, blk = b // NBLK, b % NBLK
        perm[n] = c * OWN + blk * P + cap[b]
        cap[b] += 1
        load[b, 0] += deg_lo[n]
        load[b, 1] += deg_hi[n]
        if cap[b] >= capacity[b]:
            score[b] = np.inf
        else:
            score[b] = load[b].max() * 4096 + load[b].sum()
    return perm


def _refine_swaps(perm, src, dst):
    """Swap nodes between cores within the same block (and same core-side,
    so source-segment membership is unchanged) to pull every core's
    per-(block, seg) load under the block's ceil target."""
    src_new = perm[src]
    deg = np.zeros((N_NODES, 2), dtype=np.int64)
    np.add.at(deg, (dst, (src_new >= SEG_BOUND).astype(np.int64)), 1)
    node_of = np.argsort(perm)  # new id -> old node
    moved = 0
    for side in (0, 1):
        cores = range(side * 4, side * 4 + 4)
        for b in range(NBLK):
            rows = LAST_ROWS if b == NBLK - 1 else P
            bins = {
                c: list(node_of[c * OWN + b * P : c * OWN + b * P + rows])
                for c in cores
            }
            load = {
                c: np.array([deg[bins[c], 0].sum(), deg[bins[c], 1].sum()])
                for c in cores
            }
            for g in (0, 1):
                tgt = 128 * int(
                    np.ceil(np.mean([load[c][g] for c in cores]) / P)
                )
                for c in cores:
                    guard = 0
                    while load[c][g] > tgt and guard < 200:
                        guard += 1
                        c2 = min(cores, key=lambda k: load[k][g])
                        if load[c2][g] >= tgt:
                            break
                        need = load[c][g] - tgt
                        room = tgt - load[c2][g]
                        # donor: largest deg_g node; receiver: smallest
                        i = max(range(len(bins[c])), key=lambda i: deg[bins[c][i], g])
                        j = min(range(len(bins[c2])), key=lambda j: deg[bins[c2][j], g])
                        d = deg[bins[c][i], g] - deg[bins[c2][j], g]
                        if d <= 0 or d > need + room:
                            break
                        bins[c][i], bins[c2][j] = bins[c2][j], bins[c][i]
                        dv = deg[bins[c2][j]] - deg[bins[c][i]]
                        load[c] = load[c] - dv
                        load[c2] = load[c2] + dv
                        moved += 1
            for c in cores:
                for s, n in enumerate(bins[c]):
                    perm[n] = c * OWN + b * P + s
    return perm


def _greedy_pack(nodes, deg_lo, deg_hi, cores, perm):
    """Quota-aware greedy packing of `nodes` into the bins of `cores`:
    penalize any bin-seg load crossing its chunk quota (1024 full / 896
    last block) hard, tiebreak on min-max load."""
    cores = list(cores)
    order = nodes[np.argsort(-(deg_lo[nodes] + deg_hi[nodes]), kind="stable")]
    nbins = len(cores) * NBLK
    cap = np.zeros(nbins, dtype=np.int64)
    load = np.zeros((nbins, 2), dtype=np.float64)
    capacity = np.full(nbins, P, dtype=np.int64)
    capacity[NBLK - 1 :: NBLK] = LAST_ROWS
    quota = np.full(nbins, 1024.0)
    quota[NBLK - 1 :: NBLK] = 896.0
    full = np.zeros(nbins, dtype=bool)
    for n in order:
        l0 = load[:, 0] + deg_lo[n]
        l1 = load[:, 1] + deg_hi[n]
        over = np.maximum(l0 - quota, 0.0) + np.maximum(l1 - quota, 0.0)
        score = over * 1e9 + np.maximum(l0, l1)
        score[full] = np.inf
        b = int(np.argmin(score))
        c, blk = cores[b // NBLK], b % NBLK
        perm[n] = c * OWN + blk * P + cap[b]
        cap[b] += 1
        load[b, 0] += deg_lo[n]
        load[b, 1] += deg_hi[n]
        if cap[b] >= capacity[b]:
            full[b] = True
    return perm


DUMP_BLK = 0  # global overflow block: its chunk count may grow, all others stay at quota


def _shed_cross_block(perm, src, dst):
    """Per core, move overflow above each block's quota into DUMP_BLK (or
    under-quota blocks) via same-core swaps. Same core => same side =>
    segment membership of all edges is unchanged."""
    deg = np.zeros((N_NODES, 2), dtype=np.int64)
    np.add.at(deg, (dst, (perm[src] >= SEG_BOUND).astype(np.int64)), 1)
    node_of = np.argsort(perm)
    quota = np.array([1024] * (NBLK - 1) + [896])
    for c in range(N_CORES):
        rows = [LAST_ROWS if b == NBLK - 1 else P for b in range(NBLK)]
        bins = [
            list(node_of[c * OWN + b * P : c * OWN + b * P + rows[b]])
            for b in range(NBLK)
        ]
        load = np.array([[deg[bn, 0].sum(), deg[bn, 1].sum()] for bn in bins])
        for g in (0, 1):
            for b in range(NBLK):
                if b == DUMP_BLK:
                    continue
                guard = 0
                while load[b, g] > quota[b] and guard < 300:
                    guard += 1
                    # receiver: under-quota block with most room, else dump
                    room = quota - load[:, g]
                    room[b] = -1
                    r = int(np.argmax(room))
                    if room[r] <= 0:
                        r = DUMP_BLK
                    i = max(range(len(bins[b])), key=lambda i: deg[bins[b][i], g])
                    j = min(range(len(bins[r])), key=lambda j: deg[bins[r][j], g])
                    d = deg[bins[b][i], g] - deg[bins[r][j], g]
                    dgo = deg[bins[b][i], 1 - g] - deg[bins[r][j], 1 - g]
                    if d <= 0:
                        break
                    if r != DUMP_BLK and (
                        load[r, g] + d > quota[r]
                        or load[r, 1 - g] + dgo > quota[r]
                    ):
                        r = DUMP_BLK
                        j = min(
                            range(len(bins[r])), key=lambda j: deg[bins[r][j], g]
                        )
                        d = deg[bins[b][i], g] - deg[bins[r][j], g]
                        dgo = deg[bins[b][i], 1 - g] - deg[bins[r][j], 1 - g]
                        if d <= 0:
                            break
                    bins[b][i], bins[r][j] = bins[r][j], bins[b][i]
                    load[b, g] -= d
                    load[b, 1 - g] -= dgo
                    load[r, g] += d
                    load[r, 1 - g] += dgo
        for b in range(NBLK):
            for s, n in enumerate(bins[b]):
                perm[n] = c * OWN + b * P + s
    return perm


def balance_permutation(edge_index):
    """Relabel nodes so per-(core, block, seg) incoming-edge counts sit
    under the 1024-edge chunk quota nearly everywhere: pass 1 fixes each
    node's core-side (=> every edge's segment is then exact), per-side
    quota-aware packing gets within ~8 of quota, cross-block shedding
    concentrates the residue into one global overflow block, and
    within-block swaps equalize cores."""
    src = edge_index[0].astype(np.int64)
    dst = edge_index[1].astype(np.int64)
    tot = np.bincount(dst, minlength=N_NODES)
    perm = _greedy_assign(tot, np.zeros_like(tot))
    lo_side = perm < SEG_BOUND
    deg_lo = np.bincount(dst[lo_side[src]], minlength=N_NODES)
    deg_hi = np.bincount(dst[~lo_side[src]], minlength=N_NODES)
    perm = np.empty(N_NODES, np.int64)
    _greedy_pack(np.where(lo_side)[0], deg_lo, deg_hi, range(4), perm)
    _greedy_pack(np.where(~lo_side)[0], deg_lo, deg_hi, range(4, 8), perm)
    for _ in range(2):
        perm = _shed_cross_block(perm, src, dst)
        perm = _refine_swaps(perm, src, dst)
    return perm


def _chunk_plan(C3, g):
    """Phase-g chunk plan: list of (block, kind, col) with kind 0=pure-even,
    1=pure-odd, 2=mixed; col = first dstloc column of the chunk (mixed
    chunks own cols col, col+1). Shared by host packing and device
    emission so the layouts cannot drift apart."""
    plan = []
    col = 0
    for b in range(NBLK):
        for _ in range(int(C3[g, 0, b])):
            plan.append((b, 0, col))
            col += 1
        for _ in range(int(C3[g, 1, b])):
            plan.append((b, 1, col))
            col += 1
        for _ in range(int(C3[g, 2, b])):
            plan.append((b, 2, col))
            col += 2
    return plan, col


def prep_graph(edge_index):
    """Partition edges by destination core, bucket by source segment,
    sort by destination block, pad to uniform per-(block, seg) chunk
    counts across cores. Returns per-core packed arrays + chunk plan.

    idx values are PAIR indices (src_local // 2); dstloc2 has two columns
    per chunk: col 2t = dst for even-parity sources (255 otherwise),
    col 2t+1 = dst for odd-parity sources."""
    src = edge_index[0].astype(np.int64)
    dst = edge_index[1].astype(np.int64)
    # degrees include the self-loops, but the self-loop contribution is
    # added locally in the epilogue (no gather needed for it)
    deg = 1.0 + np.bincount(dst, minlength=N_NODES).astype(np.float32)
    dis = (1.0 / np.sqrt(np.maximum(deg, 1.0))).astype(np.float32)

    nseg = 2
    segb = [0, SEG_BOUND, N_NODES]

    # per (core, block, seg) edge lists; seg 0 = the core's OWN side
    # (pair ids local to that side), seg 1 = the other side with GLOBAL
    # pair ids (matches the [lo|hi] layout of the exchanged table).
    lists = {}
    core_of = dst // OWN
    for c in range(N_CORES):
        own_base = (c // 4) * SEG_BOUND
        m = core_of == c
        s_c = src[m]
        d_c = dst[m] - c * OWN
        blk = d_c // P
        for b in range(NBLK):
            mb = blk == b
            s_b = s_c[mb]
            d_b = d_c[mb] - b * P
            own = (s_b >= own_base) & (s_b < own_base + SEG_BOUND)
            lists[(c, b, 0)] = (s_b[own] - own_base, d_b[own])
            lists[(c, b, 1)] = (s_b[~own], d_b[~own])

    # chunk plan per (seg, block): C[g] = [pure-even, pure-odd, mixed]
    # chunk counts, uniform across cores. Pure quotas are the min over
    # cores of floor(count/128) so every core fills them exactly; the
    # remainder (surplus even + surplus odd + padding) goes into mixed
    # chunks, whose count is chosen so the TOTAL equals the old
    # parity-agnostic quota: ceil((a+b)/128) == floor(a/128)+floor(b/128)
    # + ceil((ra+rb)/128), so no chunk inflation ever.
    C = np.zeros((nseg, 3, NBLK), dtype=np.int64)
    for g in range(nseg):
        for b in range(NBLK):
            ne = [int((lists[(c, b, g)][0] % 2 == 0).sum())
                  for c in range(N_CORES)]
            no = [int((lists[(c, b, g)][0] % 2 == 1).sum())
                  for c in range(N_CORES)]
            ctot = max(
                max((e + o + P - 1) // P for e, o in zip(ne, no)), 1
            )
            ce = min(e // P for e in ne)
            co = min(o // P for o in no)
            C[g, 0, b] = ce
            C[g, 1, b] = co
            C[g, 2, b] = ctot - ce - co

    # pack per core: idx per seg (chunk-plan order), dstloc columns per
    # _chunk_plan (1 col per pure chunk, 2 per mixed)
    per_core = []
    for c in range(N_CORES):
        idx_segs = []
        dl_parts = []
        for g in range(nseg):
            flat = []
            for b in range(NBLK):
                s_l, d_l = lists[(c, b, g)]
                pe = (s_l % 2) == 0
                se, de = s_l[pe], d_l[pe]
                so, do_ = s_l[~pe], d_l[~pe]
                ne = int(C[g, 0, b]) * P
                no = int(C[g, 1, b]) * P
                ms = np.concatenate([se[ne:], so[no:]]) // 2
                md = np.concatenate([de[ne:], do_[no:]]).astype(np.float32)
                mp = np.concatenate(
                    [np.zeros(len(se) - ne, np.int64),
                     np.ones(len(so) - no, np.int64)]
                )
                n_pad = int(C[g, 2, b]) * P - len(ms)
                flat += [se[:ne] // 2, so[:no] // 2, ms,
                         np.zeros(n_pad, dtype=np.int64)]
                dl_parts.append(de[:ne].astype(np.float32))
                dl_parts.append(do_[:no].astype(np.float32))
                mdp = np.concatenate([md, np.full(n_pad, 255.0, np.float32)])
                mpp = np.concatenate([mp, np.full(n_pad, -1, np.int64)])
                for t in range(int(C[g, 2, b])):
                    sd = mdp[t * P : (t + 1) * P]
                    sp = mpp[t * P : (t + 1) * P]
                    dl_parts.append(np.where(sp == 0, sd, 255.0))
                    dl_parts.append(np.where(sp == 1, sd, 255.0))
            idx_segs.append(_wrap_idx(np.concatenate(flat)))
        dl = np.concatenate(dl_parts).reshape(-1, P)  # [n_cols, P]
        dstloc = np.ascontiguousarray(dl.T).astype(BF16)  # [128, n_cols]
        per_core.append((idx_segs, dstloc))

    return dis, segb, C, per_core


def golden_aggregate(table, idx_segs, dstloc, segb, C):
    """Numpy mirror of the on-device aggregation (for packing validation).
    table: [N_NODES, F_TAB] (host dtype); returns agg [NBLK*P, F_TAB]."""
    nseg = C.shape[0]
    tab = np.asarray(table, np.float32)
    tabp = np.concatenate([tab, np.zeros((48, F_TAB), np.float32)])
    pairs = tabp.reshape(-1, 2 * F_TAB)  # [25024, 128]
    agg = np.zeros((NBLK * P, F_TAB), np.float32)
    dl = np.asarray(dstloc, np.float32)
    t_ph = 0
    for g in range(nseg):
        w = idx_segs[g][:16, :]  # [16, T*8]
        flat_idx = w.T.reshape(-1)  # idx j at [j%16, j//16]
        seg = pairs[segb[g] // 2 :]
        t0 = 0
        for b in range(NBLK):
            for t in range(C[g, b]):
                tg = t_ph + t0 + t  # global chunk col
                rows = seg[flat_idx[(t0 + t) * P : (t0 + t + 1) * P]]  # [128,128]
                iota = np.arange(P)[None, :]
                sel_e = (dl[:, 2 * tg][:, None] == iota).astype(np.float32)
                sel_o = (dl[:, 2 * tg + 1][:, None] == iota).astype(np.float32)
                agg[b * P : (b + 1) * P] += (
                    sel_e.T @ rows[:, :F_TAB] + sel_o.T @ rows[:, F_TAB:]
                )
            t0 += C[g, b]
        t_ph += t0
    return agg


def build_program(C, segb, t_tot):
    import concourse.bacc as bacc
    import concourse.mybir as mybir
    import concourse.tile as tile
    from concourse.masks import make_identity

    f32 = mybir.dt.float32
    bf16 = mybir.dt.bfloat16
    i16 = mybir.dt.int16
    nseg = C.shape[0]
    n_idx = [int(C[g].sum()) * 8 for g in range(nseg)]  # idx free dim per seg
    NPAIR = SEG_BOUND // 2  # 12500 pairs per segment

    nc = bacc.Bacc(
        "TRN2",
        num_devices=1 if SIM_1CORE else N_CORES,
        num_swdge_queues=N_QUEUES,
        dynamic_dma_scratch_size=DMA_SCRATCH,
    )

    # ---- I/O ----
    xT = nc.dram_tensor("xT", [F_IN, OWN], f32, kind="ExternalInput")
    W1 = nc.dram_tensor("W1", [F_IN, F_TAB], f32, kind="ExternalInput")
    W2 = nc.dram_tensor("W2", [F_TAB, F_TAB], f32, kind="ExternalInput")
    W3 = nc.dram_tensor("W3", [F_TAB, F_TAB], f32, kind="ExternalInput")
    b1 = nc.dram_tensor("b1", [P, F_TAB], f32, kind="ExternalInput")
    b2 = nc.dram_tensor("b2", [P, F_TAB], f32, kind="ExternalInput")
    b3 = nc.dram_tensor("b3", [P, F_TAB], f32, kind="ExternalInput")
    dis_own = nc.dram_tensor("dis_own", [P, NBLK], f32, kind="ExternalInput")
    iota_d = nc.dram_tensor("iota", [P, P], bf16, kind="ExternalInput")
    idx_d = [
        nc.dram_tensor(f"idx{g}", [P, n_idx[g]], i16, kind="ExternalInput")
        for g in range(nseg)
    ]
    n_cols = sum(_chunk_plan(C, g)[1] for g in range(nseg))
    dstloc_d = nc.dram_tensor("dstloc", [P, n_cols], bf16, kind="ExternalInput")
    out = nc.dram_tensor("out", [OWN, F_OUT], f32, kind="ExternalOutput")

    # ---- internal DRAM ----
    ag_in = nc.dram_tensor("ag_in", [OWN, F_TAB], bf16)
    # tabA[l]: this core's SIDE of the table (quad AllGather output);
    # tabF[l]: full [lo|hi] table (AllToAll exchange output).
    tabA = [
        nc.dram_tensor(f"tabA{l}", [SEG_BOUND + 48, F_TAB], bf16)
        for l in range(3)
    ]
    tabF = [
        nc.dram_tensor(f"tabF{l}", [N_NODES + 48, F_TAB], bf16)
        for l in range(3)
    ]

    W_d = [W1, W2, W3]
    bias_d = [b1, b2, b3]

    with tile.TileContext(nc) as tc:
        with (
            tc.tile_pool(name="const", bufs=1) as const_pool,
            tc.tile_pool(name="xt", bufs=1) as xt_pool,
            tc.tile_pool(name="hbuf", bufs=1) as h_pool,
            tc.tile_pool(name="hrelu", bufs=2) as hr_pool,
            tc.tile_pool(name="msg", bufs=MSG_BUFS) as msg_pool,
            tc.tile_pool(name="sel", bufs=3) as sel_pool,
            tc.tile_pool(name="small", bufs=3) as small_pool,
        ):
            # ---- preload constants ----
            ident = const_pool.tile([P, P], f32, tag="ident")
            make_identity(nc, ident[:])
            iota_sb = const_pool.tile([P, P], bf16, tag="iota")
            nc.sync.dma_start(out=iota_sb[:], in_=iota_d[:])
            dis_sb = const_pool.tile([P, NBLK], f32, tag="dis")
            nc.sync.dma_start(out=dis_sb[:], in_=dis_own[:])
            W_sb = []
            for l in range(3):
                k = F_IN if l == 0 else F_TAB
                w_t = const_pool.tile([k, F_TAB], f32, tag=f"w{l}")
                nc.sync.dma_start(out=w_t[:], in_=W_d[l][:])
                W_sb.append(w_t)
            bias_sb = []
            for l in range(3):
                b_t = const_pool.tile([P, F_TAB], f32, tag=f"b{l}")
                nc.sync.dma_start(out=b_t[:], in_=bias_d[l][:])
                bias_sb.append(b_t)
            idx_sb = []
            for g in range(nseg):
                t_i = const_pool.tile([P, n_idx[g]], i16, tag=f"idx{g}")
                nc.sync.dma_start(out=t_i[:], in_=idx_d[g][:])
                idx_sb.append(t_i)
            dstloc_sb = const_pool.tile([P, n_cols], bf16, tag="dstloc")
            nc.sync.dma_start(out=dstloc_sb[:], in_=dstloc_d[:])
            xt_sb = xt_pool.tile([F_IN, NBLK * P], f32, tag="xt")
            if OWN < NBLK * P:
                nc.vector.memset(xt_sb[:, OWN:], 0.0)
            nc.sync.dma_start(out=xt_sb[:, :OWN], in_=xT[:])

            h_cur = None  # [P, NBLK, F_TAB] f32 own rows (post-relu)
            for l in range(3):
                # ======== transform own rows -> dis * (H @ W) ====
                # t_own (f32, for self-loop epilogue) + agp (bf16 table shard)
                t_own = h_pool.tile(
                    [P, NBLK, F_TAB], f32, tag="town", name=f"town_{l}"
                )
                agp = h_pool.tile(
                    [P, NBLK, F_TAB], bf16, tag="agp", name=f"agp_{l}"
                )
                with tc.tile_pool(name="tf", bufs=2, space="PSUM") as tf_pool:
                    for k in range(N_BANK):
                        nb = min(8, NBLK - 8 * k)
                        mmps = tf_pool.tile([P, 8, F_TAB], f32, tag="mm")
                        if l > 0:
                            # transposes for this bank's blocks, staged 4/tile
                            lhsT_sb = small_pool.tile(
                                [F_TAB, 8 * P], f32, tag="lhsT"
                            )
                            for q0 in range(0, nb, 4):
                                qn = min(4, nb - q0)
                                trps = tf_pool.tile(
                                    [F_TAB, 4 * P], f32, tag="tr"
                                )
                                for j in range(qn):
                                    b = 8 * k + q0 + j
                                    nc.tensor.transpose(
                                        out=trps[:, j * P : (j + 1) * P],
                                        in_=h_cur[:, b, :],
                                        identity=ident[:],
                                    )
                                nc.scalar.copy(
                                    out=lhsT_sb[:, q0 * P : (q0 + qn) * P],
                                    in_=trps[:, : qn * P],
                                )
                        for j in range(nb):
                            b = 8 * k + j
                            if l == 0:
                                lhsT = xt_sb[:, b * P : (b + 1) * P]
                            else:
                                lhsT = lhsT_sb[:, j * P : (j + 1) * P]
                            nc.tensor.matmul(
                                out=mmps[:, j, :],
                                lhsT=lhsT,
                                rhs=W_sb[l][:],
                                start=True,
                                stop=True,
                            )
                        # batched scale by dis (per-block per-partition)
                        nc.vector.tensor_tensor(
                            out=t_own[:, 8 * k : 8 * k + nb, :],
                            in0=mmps[:, :nb, :],
                            in1=dis_sb[:, 8 * k : 8 * k + nb, None].to_broadcast(
                                [P, nb, F_TAB]
                            ),
                            op=mybir.AluOpType.mult,
                        )
                        nc.scalar.copy(
                            out=agp[:, 8 * k : 8 * k + nb, :],
                            in_=t_own[:, 8 * k : 8 * k + nb, :],
                        )
                # packed shard -> ag_in (2 strided DMAs: full blocks + tail)
                nc.sync.dma_start(
                    out=ag_in[: (NBLK - 1) * P, :]
                    .rearrange("(b p) f -> p b f", p=P),
                    in_=agp[:, : NBLK - 1, :],
                )
                nc.sync.dma_start(
                    out=ag_in[(NBLK - 1) * P :, :]
                    .rearrange("(b p) f -> p b f", p=LAST_ROWS),
                    in_=agp[:LAST_ROWS, NBLK - 1 : NBLK, :],
                )
                # ======== halo exchange, stage 1: own side ========
                if SIM_1CORE:
                    nc.sync.dma_start(out=tabA[l][:OWN, :], in_=ag_in[:])
                    nc.sync.dma_start(out=tabF[l][:OWN, :], in_=ag_in[:])
                elif not SKIP_AG:
                    nc.gpsimd.collective_compute(
                        "AllGather",
                        mybir.AluOpType.bypass,
                        replica_groups=[[0, 1, 2, 3], [4, 5, 6, 7]],
                        ins=[ag_in[:].opt()],
                        outs=[tabA[l][:SEG_BOUND, :].opt()],
                    )
                # pair views: [*, 128] bf16 (256B rows)
                pairsA = tabA[l][:].rearrange("(a b) c -> a (b c)", b=2)
                pairsF = tabF[l][:].rearrange("(a b) c -> a (b c)", b=2)
                # ======== aggregate into per-block PSUM ========
                with tc.tile_pool(name="banks", bufs=1, space="PSUM") as bk_pool:
                    banks = [
                        bk_pool.tile(
                            [P, 8, F_TAB], f32, tag=f"bank{k}", name=f"bank{k}_{l}"
                        )
                        for k in range(N_BANK)
                    ]
                    agg_lo = None
                    col_base = 0
                    call_no = 0
                    for g in range(nseg):
                        if g == 0:
                            in_view = pairsA[: NPAIR + 12, :]
                        else:
                            in_view = pairsF[: 2 * NPAIR + 12, :]
                        plan, ncols_g = _chunk_plan(C, g)
                        n_ch = len(plan)
                        tot_b = C[g].sum(axis=0)  # total chunks per block
                        ci = np.zeros(NBLK, dtype=np.int64)
                        for call0 in range(0, n_ch, G_CALL):
                            gcnt = min(G_CALL, n_ch - call0)
                            msg = msg_pool.tile(
                                [P, G_CALL, 2 * F_TAB], bf16, tag="msg"
                            )
                            nc.gpsimd.dma_gather(
                                out_ap=msg[:, :gcnt, :],
                                in_ap=in_view,
                                idxs_ap=idx_sb[g][:, call0 * 8 : (call0 + gcnt) * 8],
                                num_idxs=gcnt * P,
                                num_idxs_reg=gcnt * P,
                                elem_size=2 * F_TAB,
                                single_packet=SINGLE_PACKET,
                                queue_num=call_no % N_QUEUES,
                            )
                            call_no += 1
                            if g == 0 and call0 == 0 and not (
                                SIM_1CORE or SKIP_AG
                            ):
                                # halo stage 2 behind the first own-side
                                # gathers: pairwise cross-side exchange
                                # (out = [lo-core side | hi-core side])
                                nc.gpsimd.collective_compute(
                                    "AllGather",
                                    mybir.AluOpType.bypass,
                                    replica_groups=[[0, 4], [1, 5], [2, 6], [3, 7]],
                                    ins=[tabA[l][:SEG_BOUND, :].opt()],
                                    outs=[tabF[l][:N_NODES, :].opt()],
                                )
                            for s0 in range(0, gcnt, G_SEL):
                                scnt = min(G_SEL, gcnt - s0)
                                first = plan[call0 + s0]
                                lastc = plan[call0 + s0 + scnt - 1]
                                col0 = first[2]
                                ncols = (
                                    lastc[2] + (2 if lastc[1] == 2 else 1)
                                    - col0
                                )
                                sel = sel_pool.tile(
                                    [P, 2 * G_SEL, P], bf16, tag="sel"
                                )
                                nc.vector.tensor_tensor(
                                    out=sel[:, :ncols, :],
                                    in0=dstloc_sb[
                                        :,
                                        col_base + col0 : col_base + col0
                                        + ncols,
                                        None,
                                    ].to_broadcast([P, ncols, P]),
                                    in1=iota_sb[:, None, :].to_broadcast(
                                        [P, ncols, P]
                                    ),
                                    op=mybir.AluOpType.is_equal,
                                )
                                for j in range(scnt):
                                    b, kind, colc = plan[call0 + s0 + j]
                                    lc = colc - col0
                                    last = bool(ci[b] == tot_b[b] - 1)
                                    if kind < 2:
                                        nc.tensor.matmul(
                                            out=banks[b // 8][:, b % 8, :],
                                            lhsT=sel[:, lc, :],
                                            rhs=msg[
                                                :,
                                                s0 + j,
                                                kind * F_TAB
                                                : (kind + 1) * F_TAB,
                                            ],
                                            start=bool(ci[b] == 0),
                                            stop=last,
                                        )
                                    else:
                                        nc.tensor.matmul(
                                            out=banks[b // 8][:, b % 8, :],
                                            lhsT=sel[:, lc, :],
                                            rhs=msg[:, s0 + j, 0:F_TAB],
                                            start=bool(ci[b] == 0),
                                            stop=False,
                                        )
                                        nc.tensor.matmul(
                                            out=banks[b // 8][:, b % 8, :],
                                            lhsT=sel[:, lc + 1, :],
                                            rhs=msg[
                                                :, s0 + j, F_TAB : 2 * F_TAB
                                            ],
                                            start=False,
                                            stop=last,
                                        )
                                    ci[b] += 1
                        col_base += ncols_g
                        if nseg == 2 and g == 0:
                            # evacuate first-phase partials to reuse banks
                            agg_lo = h_pool.tile(
                                [P, NBLK, F_TAB], f32, tag="agglo",
                                name=f"agglo_{l}",
                            )
                            for k in range(N_BANK):
                                nb = min(8, NBLK - 8 * k)
                                nc.scalar.copy(
                                    out=agg_lo[:, 8 * k : 8 * k + nb, :],
                                    in_=banks[k][:, :nb, :],
                                )
                    # ======== batched epilogue ========
                    hsum = h_pool.tile(
                        [P, NBLK, F_TAB], f32, tag="hsum", name=f"hsum_{l}"
                    )
                    for k in range(N_BANK):
                        nb = min(8, NBLK - 8 * k)
                        nc.vector.tensor_tensor(
                            out=hsum[:, 8 * k : 8 * k + nb, :],
                            in0=agg_lo[:, 8 * k : 8 * k + nb, :],
                            in1=banks[k][:, :nb, :],
                            op=mybir.AluOpType.add,
                        )
                    nc.vector.tensor_tensor(
                        out=hsum[:, :, :],
                        in0=hsum[:, :, :],
                        in1=t_own[:, :, :],
                        op=mybir.AluOpType.add,
                    )
                    nc.vector.tensor_tensor(
                        out=hsum[:, :, :],
                        in0=hsum[:, :, :],
                        in1=dis_sb[:, :, None].to_broadcast([P, NBLK, F_TAB]),
                        op=mybir.AluOpType.mult,
                    )
                    nc.vector.tensor_tensor(
                        out=hsum[:, :, :],
                        in0=hsum[:, :, :],
                        in1=bias_sb[l][:, None, :].to_broadcast([P, NBLK, F_TAB]),
                        op=mybir.AluOpType.add,
                    )
                    if l < 2:
                        h_next = hr_pool.tile(
                            [P, NBLK, F_TAB], f32, tag="h", name=f"h_{l}"
                        )
                        # per-bank relu: subtile deps let the next layer's
                        # per-bank transform start as each bank lands
                        for k in range(N_BANK):
                            nb = min(8, NBLK - 8 * k)
                            nc.vector.tensor_scalar(
                                out=h_next[:, 8 * k : 8 * k + nb, :],
                                in0=hsum[:, 8 * k : 8 * k + nb, :],
                                scalar1=0.0,
                                scalar2=None,
                                op0=mybir.AluOpType.max,
                            )
                        h_cur = h_next
                    else:
                        nc.sync.dma_start(
                            out=out[: (NBLK - 1) * P, :]
                            .rearrange("(b p) f -> p b f", p=P),
                            in_=hsum[:, : NBLK - 1, :F_OUT],
                        )
                        nc.sync.dma_start(
                            out=out[(NBLK - 1) * P :, :]
                            .rearrange("(b p) f -> p b f", p=LAST_ROWS),
                            in_=hsum[:LAST_ROWS, NBLK - 1 : NBLK, :F_OUT],
                        )

    nc.compile()
    return nc


def make_in_maps(x, W1, b1, W2, b2, W3, b3, dis, C, per_core):
    W3p = np.zeros((F_TAB, F_TAB), np.float32)
    W3p[:, :F_OUT] = np.asarray(W3, np.float32)
    b3p = np.zeros((F_TAB,), np.float32)
    b3p[:F_OUT] = np.asarray(b3, np.float32)
    iota = np.broadcast_to(
        np.arange(P, dtype=np.float32), (P, P)
    ).astype(BF16).copy()

    in_maps = []
    for c in range(N_CORES):
        idx_segs, dstloc = per_core[c]
        d_own = dis[c * OWN : (c + 1) * OWN]
        pad = np.concatenate([d_own, np.ones(NBLK * P - OWN, np.float32)])
        m = {
            "xT": np.ascontiguousarray(x[c * OWN : (c + 1) * OWN].T),
            "W1": np.asarray(W1, np.float32),
            "W2": np.asarray(W2, np.float32),
            "W3": W3p,
            "b1": np.broadcast_to(np.asarray(b1, np.float32), (P, F_TAB)).copy(),
            "b2": np.broadcast_to(np.asarray(b2, np.float32), (P, F_TAB)).copy(),
            "b3": np.broadcast_to(b3p, (P, F_TAB)).copy(),
            "dis_own": np.ascontiguousarray(pad.reshape(NBLK, P).T),
            "iota": iota,
            "dstloc": dstloc,
        }
        for g in range(C.shape[0]):
            m[f"idx{g}"] = idx_segs[g]
        in_maps.append(m)
    return in_maps


_CACHE = {}


def kernel(x, edge_index, W1, b1, W2, b2, W3, b3):
    from concourse import bass_utils

    x = np.asarray(x, dtype=np.float32)
    edge_index = np.asarray(edge_index)
    key = hash(edge_index.tobytes())
    if key in _CACHE:
        nc, dis, segb, C, per_core, perm = _CACHE[key]
    else:
        perm = balance_permutation(edge_index)
        edge_perm = perm[np.asarray(edge_index, dtype=np.int64)]
        dis, segb, C, per_core = prep_graph(edge_perm)
        nc = build_program(C, segb, int(C.sum()))
        _CACHE[key] = (nc, dis, segb, C, per_core, perm)
    inv = np.argsort(perm)
    x = x[inv]  # x in new-id row order

    in_maps = make_in_maps(x, W1, b1, W2, b2, W3, b3, dis, C, per_core)

    res = bass_utils.run_bass_kernel_spmd(
        nc, in_maps, core_ids=list(range(N_CORES))
    )
    out = np.concatenate([res.results[c]["out"] for c in range(N_CORES)], axis=0)
    return out[perm]  # back to original node order

